# revision 51
# baseline (speedup 1.0000x reference)
"""Trainium2 Bass kernel for nn_Attention_8358006358422.

Reference computation (B=64, V=8, D=1024):
    BN over all B*V rows per feature d -> img
    x_qk = qk_w @ img ; x_v = v_w @ img + bias
    energy[b] = x_qk[b]^T x_qk[b]  (D x D, contraction over V)
    att = softmax(energy, -1); att /= (1e-9 + sum(att, axis=1))
    out = img + x_v @ att

Kernel strategy (8 NeuronCores, data-parallel over B, 8 batches/core):
  * BN stats reduced redundantly per core via ones-vector matmuls; the
    BN "+beta2" offset never materializes: it enters x_qk / x_v as
    rank-1 K=1 matmul accumulations (s (x) beta2, vb (x) ones).
  * energy row-blocks [128, 1024] -> exp on ACT; the free-dim
    accumulator gives rowsum [128, 1] per block for free.
  * attention application runs transposed: yT[e, ch] accumulates
    esb_kj^T @ xaug_k with a folded ones-channel producing the column
    renorm sums, so renorm+residual are [128, 64]-shaped DVE ops.
  * the residual img is rebuilt transposed from a host-pretransposed
    featT shard; the output is stored transposed and the host undoes
    the transpose for free.
"""

import sys
import numpy as np

sys.path.insert(0, "/opt/trn_rl_repo")

B, V, D = 64, 8, 1024
NCORES = 8
BPC = B // NCORES          # batches per core
ROWS = B * V               # 512 BN rows
NBLK = D // 128            # 8 d-blocks of 128
BN_EPS = 1e-5
ESB_RING = 16

_BUILT = None


def _build_program():
    import concourse.bass as bass
    import concourse.mybir as mybir
    import concourse.tile as tile
    from concourse import bacc
    from concourse.hw_specs import get_activation_tables
    from contextlib import ExitStack

    fp32 = mybir.dt.float32
    F32R = mybir.dt.float32r
    MULT = mybir.AluOpType.mult
    ADD = mybir.AluOpType.add
    SUB = mybir.AluOpType.subtract
    EXP = mybir.ActivationFunctionType.Exp
    LN = mybir.ActivationFunctionType.Ln
    SQUARE = mybir.ActivationFunctionType.Square

    nc = bacc.Bacc(
        "TRN2",
        target_bir_lowering=False,
        debug=False,
        enable_asserts=False,
        num_devices=NCORES,
    )

    # ---- DRAM I/O ----
    feat_full = nc.dram_tensor("feat_full", [ROWS, D], F32R, kind="ExternalInput")
    feat_shard = nc.dram_tensor("feat_shard", [BPC * V, D], F32R, kind="ExternalInput")
    featT_shard = nc.dram_tensor("featT_shard", [128, BPC * NBLK * V], F32R,
                                 kind="ExternalInput")
    gamma_d = nc.dram_tensor("gamma", [1, D], fp32, kind="ExternalInput")
    beta_d = nc.dram_tensor("beta", [1, D], fp32, kind="ExternalInput")
    wq8_d = nc.dram_tensor("wq8T", [V, V], F32R, kind="ExternalInput")
    wqr8_d = nc.dram_tensor("wq_r8", [1, V], F32R, kind="ExternalInput")
    vw8_d = nc.dram_tensor("vw8T", [V, 10], F32R, kind="ExternalInput")
    vwr8_d = nc.dram_tensor("vw_r8", [1, 10], F32R, kind="ExternalInput")
    vwr9_d = nc.dram_tensor("vw_r9", [1, 10], F32R, kind="ExternalInput")
    outT_d = nc.dram_tensor("outT", [128, BPC * NBLK * V], fp32,
                            kind="ExternalOutput")

    with tile.TileContext(nc) as tc, ExitStack() as ctx:
        const = ctx.enter_context(tc.tile_pool(name="const", bufs=1))
        ftp = ctx.enter_context(tc.tile_pool(name="ftp", bufs=4))
        sqp = ctx.enter_context(tc.tile_pool(name="sqp", bufs=8))
        ybp = ctx.enter_context(tc.tile_pool(name="ybp", bufs=3))
        xgp = ctx.enter_context(tc.tile_pool(name="xgp", bufs=3))
        esbp = ctx.enter_context(tc.tile_pool(name="esbp", bufs=ESB_RING))
        finp = ctx.enter_context(tc.tile_pool(name="finp", bufs=3))

        pe_pool = ctx.enter_context(tc.tile_pool(name="pe", bufs=2, space="PSUM"))
        pq_pool = ctx.enter_context(tc.tile_pool(name="pq", bufs=1, space="PSUM"))
        ps_pool = ctx.enter_context(tc.tile_pool(name="ps", bufs=2, space="PSUM"))

        # explicit natural_log_exp table load so Ln and Exp share one set
        tables = list(get_activation_tables(nc.m.arch).keys())
        if "natural_log_exp_and_others" in tables:
            nc.scalar.add_instruction(mybir.InstLoadActFuncSet(
                name=nc.get_next_instruction_name(),
                ins=[], outs=[],
                act_func_set_id=tables.index("natural_log_exp_and_others")))

        # ---- constants ----
        ones_col = const.tile([128, 2], fp32)
        nc.vector.memset(ones_col[:], 1.0)
        ones_col_r = const.tile([128, 2], F32R)
        nc.vector.tensor_scalar_mul(ones_col_r[:], ones_col[:], 1.0)
        ones128x8 = const.tile([128, V], fp32)
        nc.vector.memset(ones128x8[:], 1.0)
        one_two_f = const.tile([1, 2], fp32)
        nc.vector.memset(one_two_f[:], 1.0)
        one_two = const.tile([1, 2], F32R)
        nc.vector.tensor_scalar_mul(one_two[:], one_two_f[:], 1.0)
        onesD_f = const.tile([1, D], fp32)
        nc.vector.memset(onesD_f[:], 1.0)
        onesD = const.tile([1, D], F32R)
        nc.vector.tensor_scalar_mul(onesD[:], onesD_f[:], 1.0)
        eps_sc = const.tile([1, 1], fp32)
        nc.vector.memset(eps_sc[:], BN_EPS)

        # ---- feat_full load + BN statistics (redundant on every core) ----
        ffull = feat_full[:, :]
        sum_ps = pe_pool.tile([1, D], fp32, tag="pe", name="sum_ps")
        sq_ps = pe_pool.tile([1, D], fp32, tag="pe", name="sq_ps")
        ft_tiles = []
        for r in range(4):
            ft = ftp.tile([128, D], F32R)
            nc.sync.dma_start(ft[:], ffull[128 * r: 128 * (r + 1), :])
            ft_tiles.append(ft)
        yb0 = ybp.tile([V, D], F32R, tag="yb", name="yb0")
        nc.sync.dma_start(yb0[:], feat_shard[0:V, :])
        gamma_sb = const.tile([1, D], fp32)
        nc.sync.dma_start(gamma_sb[:], gamma_d[:, :])
        beta_sb = const.tile([1, D], fp32)
        nc.sync.dma_start(beta_sb[:], beta_d[:, :])
        wq8_sb = const.tile([V, V], F32R)
        nc.sync.dma_start(wq8_sb[:], wq8_d[:, :])
        wqr8_sb = const.tile([1, V], F32R)
        nc.sync.dma_start(wqr8_sb[:], wqr8_d[:, :])
        vw8_sb = const.tile([V, 10], F32R)
        nc.sync.dma_start(vw8_sb[:], vw8_d[:, :])
        vwr8_sb = const.tile([1, 10], F32R)
        nc.sync.dma_start(vwr8_sb[:], vwr8_d[:, :])
        vwr9_sb = const.tile([1, 10], F32R)
        nc.sync.dma_start(vwr9_sb[:], vwr9_d[:, :])
        featT_sb = const.tile([128, BPC * NBLK * V], F32R)
        nc.sync.dma_start(featT_sb[:], featT_shard[:, :])

        # h-major square+stat matmuls: half-0 stats complete early so the
        # half-0 chain (and the first exp) starts sooner. Squares run on the
        # otherwise-idle Pool engine.
        for h in range(2):
            c = slice(512 * h, 512 * (h + 1))
            for r in range(4):
                ft = ft_tiles[r]
                sq = sqp.tile([128, 512], F32R)
                nc.gpsimd.tensor_tensor(sq[:], ft[:, c], ft[:, c], op=MULT)
                st = r == 0
                sp = r == 3
                nc.tensor.matmul(sum_ps[0:1, c], ones_col_r[:, 0:1],
                                 ft[:, c], start=st, stop=sp)
                nc.tensor.matmul(sq_ps[0:1, c], ones_col_r[:, 0:1],
                                 sq[:], start=st, stop=sp)

        # mean, var, alpha=gamma*rstd, beta2=beta-mean*alpha. Emitted h0
        # fully (chain + batch-0 im8/xq/copy) before any h1 work so the DVE
        # stream drives the first exp with no h1 interleaving.
        mean_sb = const.tile([1, D], fp32)
        msq = const.tile([1, D], fp32)
        vpe = const.tile([1, D], fp32)
        rv = const.tile([1, D], fp32)
        rstd = const.tile([1, D], fp32)
        alpha_row = const.tile([1, D], F32R)
        tmp_row = const.tile([1, D], fp32)
        beta2_row = const.tile([1, D], F32R)
        alphaB = const.tile([V, D], fp32)

        def chain_h(h):
            c = slice(512 * h, 512 * (h + 1))
            nc.vector.tensor_scalar_mul(mean_sb[:, c], sum_ps[0:1, c], 1.0 / ROWS)
            nc.vector.tensor_mul(msq[:, c], mean_sb[:, c], mean_sb[:, c])
            nc.vector.scalar_tensor_tensor(vpe[:, c], sq_ps[0:1, c], 1.0 / ROWS,
                                           msq[:, c], op0=MULT, op1=SUB)
            # rstd = (var+eps)^-0.5 via exp(-0.5*ln(var+eps)): one table set
            nc.scalar.activation(rv[:, c], vpe[:, c], LN, bias=eps_sc[:])
            nc.scalar.activation(rstd[:, c], rv[:, c], EXP, scale=-0.5)
            nc.vector.tensor_mul(alpha_row[:, c], gamma_sb[:, c], rstd[:, c])
            nc.gpsimd.partition_broadcast(alphaB[:, c],
                                          alpha_row[:, c].bitcast(fp32))
            nc.vector.tensor_mul(tmp_row[:, c],
                                 mean_sb[:, c], alpha_row[:, c].bitcast(fp32))
            nc.vector.tensor_sub(beta2_row[:, c], beta_sb[:, c], tmp_row[:, c])

        # transposed alpha/beta2 via K=1 matmuls (N=2 for ISA legality); kept
        # as [128, 16] (a j / b j) for broadcast-AP reads on the imgT path.
        # Emitted lazily (inside batch 0) to stay off the startup path.
        atb_sb = const.tile([128, 32], fp32)

        def build_atb():
            atb_ps = ps_pool.tile([128, 512], fp32, tag="ps", name="atb_ps")
            for j in range(NBLK):
                cb = slice(128 * j, 128 * (j + 1))
                nc.tensor.matmul(atb_ps[:, 2 * j:2 * j + 2],
                                 alpha_row[:, cb], one_two[:],
                                 start=True, stop=True)
                nc.tensor.matmul(atb_ps[:, 16 + 2 * j:18 + 2 * j],
                                 beta2_row[:, cb], one_two[:],
                                 start=True, stop=True)
            nc.vector.tensor_copy(atb_sb[:], atb_ps[:, 0:32])

        fshard = feat_shard[:, :]
        outT_ap = outT_d[:, :]

        state = {}

        def prepare(b):
            """DMA y_b; im8 = y*alpha on DVE."""
            yb = ybp.tile([V, D], F32R, tag="yb", name=f"yb{b}")
            nc.sync.dma_start(yb[:], fshard[V * b: V * (b + 1), :])
            im8 = ybp.tile([V, D], F32R, tag="im8", name=f"im8_{b}")
            nc.vector.tensor_mul(im8[:], yb[:], alphaB[:, :].bitcast(F32R))
            rs8 = finp.tile([128, NBLK], fp32, tag="rs8", name=f"rs8_{b}")
            state[b] = [im8, None, [None] * NBLK, None, None, rs8]

        def prepare_xq(b):
            """xq = Wq@im8 + s(x)beta2 -> SBUF f32r."""
            im8 = state[b][0]
            xq_ps = pq_pool.tile([V, D], fp32, tag="pq", name=f"xq{b}")
            xg = xgp.tile([V, D], F32R, tag="xg", name=f"xg{b}")
            for h in range(2):
                c = slice(512 * h, 512 * (h + 1))
                nc.tensor.matmul(xq_ps[:, c], wq8_sb[:], im8[:, c],
                                 start=True, stop=False)
                nc.tensor.matmul(xq_ps[:, c], wqr8_sb[:], beta2_row[:, c],
                                 start=False, stop=True)
            nc.vector.tensor_copy(xg[:], xq_ps[:])
            state[b][1] = xg

        def prepare_xv(b):
            """xvT via 3 matmuls per block (beta2 + bias/ones rank-1 folds).
            sm layout: [0:80] xvT 10-col groups, [80:160] yT 10-col groups."""
            im8 = state[b][0]
            sm = ps_pool.tile([128, 512], fp32, tag="ps", name=f"sm{b}")
            for k in range(NBLK):
                dblk = slice(128 * k, 128 * (k + 1))
                g = slice(10 * k, 10 * (k + 1))
                nc.tensor.matmul(sm[:, g], im8[:, dblk], vw8_sb[:],
                                 start=True, stop=False)
                nc.tensor.matmul(sm[:, g], beta2_row[:, dblk], vwr8_sb[:],
                                 start=False, stop=False)
                nc.tensor.matmul(sm[:, g], onesD[:, dblk], vwr9_sb[:],
                                 start=False, stop=True)
            state[b][3] = sm

        rsx = const.tile([128, 2], fp32)

        def energy(b, k):
            xg = state[b][1]
            rs8 = state[b][5]
            dblk = slice(128 * k, 128 * (k + 1))
            pe = pe_pool.tile([128, D], fp32, tag="pe", name=f"pe{b}_{k}")
            esb = esbp.tile([128, D], F32R, tag="esb", name=f"esb{b}_{k}")
            if b == 0 and k < 2:
                # batch-0 head: half-width energy+exp so ACT starts on the
                # h0 chain without waiting for the h1 BN chain
                for h in range(2):
                    c = slice(512 * h, 512 * (h + 1))
                    nc.tensor.matmul(pe[:, c], xg[:, dblk], xg[:, c],
                                     start=True, stop=True)
                    acc = rs8[:, k:k + 1] if h == 0 else rsx[:, k:k + 1]
                    nc.scalar.activation(esb[:, c], pe[:, c], EXP,
                                         accum_out=acc)
                if k == 1:
                    nc.vector.tensor_add(rs8[:, 0:2], rs8[:, 0:2], rsx[:])
            else:
                for h in range(2):
                    c = slice(512 * h, 512 * (h + 1))
                    nc.tensor.matmul(pe[:, c], xg[:, dblk], xg[:, c],
                                     start=True, stop=True)
                nc.scalar.activation(esb[:], pe[:, :], EXP,
                                     accum_out=rs8[:, k:k + 1])
            state[b][2][k] = esb

        def bcast3(ap3, like3):
            from concourse.bass import broadcast_tensor_aps
            _, b3 = broadcast_tensor_aps(like3, ap3)
            return b3

        def xaug(b):
            """recip of rowsum; xaug = xvT * recip, one broadcast TT."""
            sm = state[b][3]
            rs8 = state[b][5]
            rr = finp.tile([128, NBLK], F32R, tag="rr", name=f"rr{b}")
            with nc.allow_low_precision(reason="f32r recip, small rel budget"):
                nc.vector.reciprocal(rr[:], rs8[:])
            xa = finp.tile([128, 10 * NBLK], F32R, tag="xa", name=f"xa{b}")
            xav = xa[:, :].rearrange("p (k c) -> p k c", c=10)
            smv = sm[:, 0:80].rearrange("p (k c) -> p k c", c=10)
            rrv = rr[:, :].bitcast(fp32).rearrange("p k -> p k ()")
            nc.vector.tensor_tensor(xav, smv, bcast3(rrv, xav), op=MULT)
            state[b][4] = xa

        def ymm(b, js):
            """yT_j[128, 10] += esb_kj^T @ xaug_k (col 8 = colsum')."""
            sm = state[b][3]
            xa = state[b][4]
            for j in js:
                eblk = slice(128 * j, 128 * (j + 1))
                for k in range(NBLK):
                    esb = state[b][2][k]
                    nc.tensor.matmul(sm[:, 80 + 10 * j: 90 + 10 * j],
                                     esb[:, eblk], xa[:, 10 * k: 10 * (k + 1)],
                                     start=(k == 0), stop=(k == NBLK - 1))

        def finalize(b, half=None):
            """s = 1/(1e-9+colsum'); osbT = yT*s + imgT; store transposed.
            half=0/1 processes j-blocks 0-3 / 4-7 (used to pipeline the
            final batch's renorm with its output DMA)."""
            sm = state[b][3]
            js = slice(0, NBLK) if half is None else slice(4 * half, 4 * half + 4)
            nj = js.stop - js.start
            if half in (None, 0):
                st = finp.tile([128, NBLK], fp32, tag="st", name=f"st{b}")
                s = finp.tile([128, NBLK], fp32, tag="s", name=f"s{b}")
                imgT = finp.tile([128, NBLK * V], fp32, tag="imgT",
                                 name=f"imgT{b}")
                osbT = finp.tile([128, NBLK * V], fp32, tag="osbT",
                                 name=f"osbT{b}")
                state[b].append((st, s, imgT, osbT))
            else:
                st, s, imgT, osbT = state[b][-1]
            cs_v = sm[:, 80:160].rearrange("p (j c) -> p j c", c=10)
            nc.vector.tensor_scalar_add(
                st[:, js].rearrange("p j -> p j ()"),
                cs_v[:, js, 8:9], 1e-9)
            nc.vector.reciprocal(s[:, js], st[:, js])
            fTv = featT_sb[:, 64 * b: 64 * (b + 1)].bitcast(fp32).rearrange(
                "p (j v) -> p j v", v=V)
            imv = imgT[:, :].rearrange("p (j v) -> p j v", v=V)
            aTv = atb_sb[:, 0:16].rearrange(
                "p (j c) -> p j c", c=2)[:, js, 0:1]
            bTv = atb_sb[:, 16:32].rearrange(
                "p (j c) -> p j c", c=2)[:, js, 0:1]
            nc.vector.tensor_tensor(imv[:, js, :], fTv[:, js, :],
                                    bcast3(aTv, imv[:, js, :]), op=MULT)
            nc.vector.tensor_tensor(imv[:, js, :], imv[:, js, :],
                                    bcast3(bTv, imv[:, js, :]), op=ADD)
            osv = osbT[:, :].rearrange("p (j v) -> p j v", v=V)
            sv3 = s[:, :].rearrange("p j -> p j ()")
            nc.vector.tensor_tensor(osv[:, js, :], cs_v[:, js, 0:8],
                                    bcast3(sv3[:, js, :], osv[:, js, :]),
                                    op=MULT)
            nc.vector.tensor_add(osv[:, js, :], osv[:, js, :], imv[:, js, :])
            nc.sync.dma_start(
                outT_ap[:, 64 * b + 8 * js.start: 64 * b + 8 * js.stop],
                osbT[:, 8 * js.start: 8 * js.stop])
            if half in (None, 1):
                state.pop(b)

        # ---- batch-0 h0-strict preamble: full h0 chain + h0 prep before
        # any h1 work, so the first exp isn't delayed by h1 chain ops ----
        im8_0 = ybp.tile([V, D], F32R, tag="im8", name="im8_0")
        xq_ps0 = pq_pool.tile([V, D], fp32, tag="pq", name="xq0")
        xg0 = xgp.tile([V, D], F32R, tag="xg", name="xg0")
        for h in range(2):
            c = slice(512 * h, 512 * (h + 1))
            chain_h(h)
            # im8 on Pool and the PSUM->SBUF copy on (idle) ACT keep the
            # startup-critical DVE stream down to the bare BN chain
            nc.gpsimd.tensor_tensor(im8_0[:, c], yb0[:, c],
                                    alphaB[:, c].bitcast(F32R), op=MULT)
            nc.tensor.matmul(xq_ps0[:, c], wq8_sb[:], im8_0[:, c],
                             start=True, stop=False)
            nc.tensor.matmul(xq_ps0[:, c], wqr8_sb[:], beta2_row[:, c],
                             start=False, stop=True)
            nc.scalar.copy(xg0[:, c], xq_ps0[:, c])
        rs8_0 = finp.tile([128, NBLK], fp32, tag="rs8", name="rs8_0")
        state[0] = [im8_0, xg0, [None] * NBLK, None, None, rs8_0]

        # ---- software-pipelined main loop ----
        for b in range(BPC):
            nxt = b + 1 if b + 1 < BPC else None
            prv = b - 1 if b > 0 else None
            for k in range(NBLK):
                energy(b, k)
                if b == 0:
                    if k == 2:
                        prepare_xv(0)
                    elif k == 5:
                        build_atb()
                if prv is not None and 0 <= k <= 3:
                    ymm(prv, range(2 * k, 2 * k + 2))
                    if k == 3:
                        finalize(prv)
                if nxt is not None:
                    if k == 0:
                        prepare(nxt)
                    elif k == 4:
                        prepare_xq(nxt)
                    elif k == 5:
                        prepare_xv(nxt)
            xaug(b)
        last = BPC - 1
        ymm(last, range(4))
        finalize(last, half=0)
        ymm(last, range(4, NBLK))
        finalize(last, half=1)

    nc.compile()
    return nc


def _get():
    global _BUILT
    if _BUILT is None:
        _BUILT = _build_program()
    return _BUILT


def _make_in_maps(inputs):
    feat = np.ascontiguousarray(np.asarray(inputs["feat"], dtype=np.float32))
    gamma = np.asarray(inputs["bn_gamma"], dtype=np.float32).reshape(1, D)
    beta = np.asarray(inputs["bn_beta"], dtype=np.float32).reshape(1, D)
    qk = np.asarray(inputs["qk_weight"], dtype=np.float32)
    vw = np.asarray(inputs["v_weight"], dtype=np.float32)
    vb = np.asarray(inputs["v_bias"], dtype=np.float32)
    wq8 = np.ascontiguousarray(qk.T)
    wq_r8 = qk.sum(axis=1).reshape(1, V)
    # vw8T columns: 0-7 real x_v channels, 8 = constant-ones channel
    # (becomes the colsum' source after the recip scale), 9 = zero pad
    vw8 = np.zeros((V, 10), dtype=np.float32)
    vw8[:, 0:V] = vw.T
    vw_r8 = np.zeros((1, 10), dtype=np.float32)
    vw_r8[0, 0:V] = vw.sum(axis=1)
    vw_r9 = np.zeros((1, 10), dtype=np.float32)
    vw_r9[0, 0:V] = vb
    vw_r9[0, V] = 1.0
    full = np.ascontiguousarray(feat.reshape(ROWS, D))
    in_maps = []
    for c in range(NCORES):
        fc = feat[BPC * c: BPC * (c + 1)]              # [8, 8, 1024]
        shard = np.ascontiguousarray(fc.reshape(BPC * V, D))
        # featT[p, b*64 + j*8 + v] = fc[b, v, j*128 + p]
        ft4 = fc.reshape(BPC, V, NBLK, 128)            # b, v, j, p
        featT = np.ascontiguousarray(
            ft4.transpose(3, 0, 2, 1).reshape(128, BPC * NBLK * V))
        in_maps.append({
            "feat_full": full,
            "feat_shard": shard,
            "featT_shard": featT,
            "gamma": gamma,
            "beta": beta,
            "wq8T": wq8,
            "wq_r8": wq_r8,
            "vw8T": vw8,
            "vw_r8": vw_r8,
            "vw_r9": vw_r9,
        })
    return in_maps


def _run(inputs, **kw):
    from concourse.bass_utils import run_bass_kernel_spmd
    nc = _get()
    res = run_bass_kernel_spmd(nc, _make_in_maps(inputs),
                               core_ids=list(range(NCORES)), **kw)
    outs = []
    for c in range(NCORES):
        oT = res.results[c]["outT"]                    # [128, 512]
        o4 = oT.reshape(128, BPC, NBLK, V)             # p, b, j, v
        outs.append(o4.transpose(1, 3, 2, 0).reshape(BPC, V, D))
    return np.concatenate(outs, axis=0), res


def kernel(**inputs) -> np.ndarray:
    out, _ = _run(inputs)
    return out


def run_profiled(inputs, **kw):
    return _run(inputs, trace=True, **kw)


# revision 53
# speedup vs baseline: 1.0082x; 1.0082x over previous
"""Trainium2 Bass kernel for nn_Attention_8358006358422.

Reference computation (B=64, V=8, D=1024):
    BN over all B*V rows per feature d -> img
    x_qk = qk_w @ img ; x_v = v_w @ img + bias
    energy[b] = x_qk[b]^T x_qk[b]  (D x D, contraction over V)
    att = softmax(energy, -1); att /= (1e-9 + sum(att, axis=1))
    out = img + x_v @ att

Kernel strategy (8 NeuronCores, data-parallel over B, 8 batches/core):
  * BN stats reduced redundantly per core via ones-vector matmuls; the
    BN "+beta2" offset never materializes: it enters x_qk / x_v as
    rank-1 K=1 matmul accumulations (s (x) beta2, vb (x) ones).
  * energy row-blocks [128, 1024] -> exp on ACT; the free-dim
    accumulator gives rowsum [128, 1] per block for free.
  * attention application runs transposed: yT[e, ch] accumulates
    esb_kj^T @ xaug_k with a folded ones-channel producing the column
    renorm sums, so renorm+residual are [128, 64]-shaped DVE ops.
  * the residual img is rebuilt transposed from a host-pretransposed
    featT shard; the output is stored transposed and the host undoes
    the transpose for free.
"""

import sys
import numpy as np

sys.path.insert(0, "/opt/trn_rl_repo")

B, V, D = 64, 8, 1024
NCORES = 8
BPC = B // NCORES          # batches per core
ROWS = B * V               # 512 BN rows
NBLK = D // 128            # 8 d-blocks of 128
BN_EPS = 1e-5
ESB_RING = 16

_BUILT = None


def _build_program():
    import concourse.bass as bass
    import concourse.mybir as mybir
    import concourse.tile as tile
    from concourse import bacc
    from concourse.hw_specs import get_activation_tables
    from contextlib import ExitStack

    fp32 = mybir.dt.float32
    F32R = mybir.dt.float32r
    MULT = mybir.AluOpType.mult
    ADD = mybir.AluOpType.add
    SUB = mybir.AluOpType.subtract
    EXP = mybir.ActivationFunctionType.Exp
    LN = mybir.ActivationFunctionType.Ln
    SQUARE = mybir.ActivationFunctionType.Square

    nc = bacc.Bacc(
        "TRN2",
        target_bir_lowering=False,
        debug=False,
        enable_asserts=False,
        num_devices=NCORES,
    )

    # ---- DRAM I/O ----
    feat_full = nc.dram_tensor("feat_full", [ROWS, D], F32R, kind="ExternalInput")
    feat_shard = nc.dram_tensor("feat_shard", [BPC * V, D], F32R, kind="ExternalInput")
    featT_shard = nc.dram_tensor("featT_shard", [128, BPC * NBLK * V], F32R,
                                 kind="ExternalInput")
    gamma_d = nc.dram_tensor("gamma", [1, D], fp32, kind="ExternalInput")
    beta_d = nc.dram_tensor("beta", [1, D], fp32, kind="ExternalInput")
    wq8_d = nc.dram_tensor("wq8T", [V, V], F32R, kind="ExternalInput")
    wqr8_d = nc.dram_tensor("wq_r8", [1, V], F32R, kind="ExternalInput")
    vw8_d = nc.dram_tensor("vw8T", [V, 10], F32R, kind="ExternalInput")
    vwr8_d = nc.dram_tensor("vw_r8", [1, 10], F32R, kind="ExternalInput")
    vwr9_d = nc.dram_tensor("vw_r9", [1, 10], F32R, kind="ExternalInput")
    outT_d = nc.dram_tensor("outT", [128, BPC * NBLK * V], fp32,
                            kind="ExternalOutput")

    with tile.TileContext(nc) as tc, ExitStack() as ctx:
        const = ctx.enter_context(tc.tile_pool(name="const", bufs=1))
        ftp = ctx.enter_context(tc.tile_pool(name="ftp", bufs=4))
        sqp = ctx.enter_context(tc.tile_pool(name="sqp", bufs=8))
        ybp = ctx.enter_context(tc.tile_pool(name="ybp", bufs=3))
        xgp = ctx.enter_context(tc.tile_pool(name="xgp", bufs=3))
        esbp = ctx.enter_context(tc.tile_pool(name="esbp", bufs=ESB_RING))
        finp = ctx.enter_context(tc.tile_pool(name="finp", bufs=3))

        pe_pool = ctx.enter_context(tc.tile_pool(name="pe", bufs=2, space="PSUM"))
        pq_pool = ctx.enter_context(tc.tile_pool(name="pq", bufs=1, space="PSUM"))
        ps_pool = ctx.enter_context(tc.tile_pool(name="ps", bufs=2, space="PSUM"))

        # explicit natural_log_exp table load so Ln and Exp share one set
        tables = list(get_activation_tables(nc.m.arch).keys())
        if "natural_log_exp_and_others" in tables:
            nc.scalar.add_instruction(mybir.InstLoadActFuncSet(
                name=nc.get_next_instruction_name(),
                ins=[], outs=[],
                act_func_set_id=tables.index("natural_log_exp_and_others")))

        # ---- constants ----
        ones_col = const.tile([128, 2], fp32)
        nc.vector.memset(ones_col[:], 1.0)
        ones_col_r = const.tile([128, 2], F32R)
        nc.vector.tensor_scalar_mul(ones_col_r[:], ones_col[:], 1.0)
        ones128x8 = const.tile([128, V], fp32)
        nc.vector.memset(ones128x8[:], 1.0)
        one_two_f = const.tile([1, 2], fp32)
        nc.vector.memset(one_two_f[:], 1.0)
        one_two = const.tile([1, 2], F32R)
        nc.vector.tensor_scalar_mul(one_two[:], one_two_f[:], 1.0)
        onesD_f = const.tile([1, D], fp32)
        nc.vector.memset(onesD_f[:], 1.0)
        onesD = const.tile([1, D], F32R)
        nc.vector.tensor_scalar_mul(onesD[:], onesD_f[:], 1.0)
        eps_sc = const.tile([1, 1], fp32)
        nc.vector.memset(eps_sc[:], BN_EPS)

        # ---- feat_full load + BN statistics (redundant on every core) ----
        ffull = feat_full[:, :]
        sum_ps = pe_pool.tile([1, D], fp32, tag="pe", name="sum_ps")
        sq_ps = pe_pool.tile([1, D], fp32, tag="pe", name="sq_ps")
        ft_tiles = []
        for r in range(4):
            ft = ftp.tile([128, D], F32R)
            nc.sync.dma_start(ft[:], ffull[128 * r: 128 * (r + 1), :])
            ft_tiles.append(ft)
        yb0 = ybp.tile([V, D], F32R, tag="yb", name="yb0")
        nc.sync.dma_start(yb0[:], feat_shard[0:V, :])
        gamma_sb = const.tile([1, D], fp32)
        nc.sync.dma_start(gamma_sb[:], gamma_d[:, :])
        beta_sb = const.tile([1, D], fp32)
        nc.sync.dma_start(beta_sb[:], beta_d[:, :])
        wq8_sb = const.tile([V, V], F32R)
        nc.sync.dma_start(wq8_sb[:], wq8_d[:, :])
        wqr8_sb = const.tile([1, V], F32R)
        nc.sync.dma_start(wqr8_sb[:], wqr8_d[:, :])
        vw8_sb = const.tile([V, 10], F32R)
        nc.sync.dma_start(vw8_sb[:], vw8_d[:, :])
        vwr8_sb = const.tile([1, 10], F32R)
        nc.sync.dma_start(vwr8_sb[:], vwr8_d[:, :])
        vwr9_sb = const.tile([1, 10], F32R)
        nc.sync.dma_start(vwr9_sb[:], vwr9_d[:, :])
        featT_sb = const.tile([128, BPC * NBLK * V], F32R)
        nc.sync.dma_start(featT_sb[:], featT_shard[:, :])

        # h-major square+stat matmuls: half-0 stats complete early so the
        # half-0 chain (and the first exp) starts sooner. Squares run on the
        # otherwise-idle Pool engine.
        for h in range(2):
            c = slice(512 * h, 512 * (h + 1))
            for r in range(4):
                ft = ft_tiles[r]
                sq = sqp.tile([128, 512], F32R)
                nc.gpsimd.tensor_tensor(sq[:], ft[:, c], ft[:, c], op=MULT)
                st = r == 0
                sp = r == 3
                nc.tensor.matmul(sum_ps[0:1, c], ones_col_r[:, 0:1],
                                 ft[:, c], start=st, stop=sp)
                nc.tensor.matmul(sq_ps[0:1, c], ones_col_r[:, 0:1],
                                 sq[:], start=st, stop=sp)

        # mean, var, alpha=gamma*rstd, beta2=beta-mean*alpha. Emitted h0
        # fully (chain + batch-0 im8/xq/copy) before any h1 work so the DVE
        # stream drives the first exp with no h1 interleaving.
        mean_sb = const.tile([1, D], fp32)
        msq = const.tile([1, D], fp32)
        vpe = const.tile([1, D], fp32)
        rv = const.tile([1, D], fp32)
        rstd = const.tile([1, D], fp32)
        alpha_row = const.tile([1, D], F32R)
        tmp_row = const.tile([1, D], fp32)
        beta2_row = const.tile([1, D], F32R)
        alphaB = const.tile([V, D], fp32)

        def chain_h(h):
            c = slice(512 * h, 512 * (h + 1))
            nc.vector.tensor_scalar_mul(mean_sb[:, c], sum_ps[0:1, c], 1.0 / ROWS)
            nc.vector.tensor_mul(msq[:, c], mean_sb[:, c], mean_sb[:, c])
            nc.vector.scalar_tensor_tensor(vpe[:, c], sq_ps[0:1, c], 1.0 / ROWS,
                                           msq[:, c], op0=MULT, op1=SUB)
            # rstd = (var+eps)^-0.5 via exp(-0.5*ln(var+eps)): one table set
            nc.scalar.activation(rv[:, c], vpe[:, c], LN, bias=eps_sc[:])
            nc.scalar.activation(rstd[:, c], rv[:, c], EXP, scale=-0.5)
            nc.vector.tensor_mul(alpha_row[:, c], gamma_sb[:, c], rstd[:, c])
            nc.gpsimd.partition_broadcast(alphaB[:, c],
                                          alpha_row[:, c].bitcast(fp32))
            nc.vector.tensor_mul(tmp_row[:, c],
                                 mean_sb[:, c], alpha_row[:, c].bitcast(fp32))
            nc.vector.tensor_sub(beta2_row[:, c], beta_sb[:, c], tmp_row[:, c])

        # transposed alpha/beta2 via K=1 matmuls (N=2 for ISA legality); kept
        # as [128, 16] (a j / b j) for broadcast-AP reads on the imgT path.
        # Emitted lazily (inside batch 0) to stay off the startup path.
        atb_sb = const.tile([128, 32], fp32)

        def build_atb():
            atb_ps = ps_pool.tile([128, 512], fp32, tag="ps", name="atb_ps")
            for j in range(NBLK):
                cb = slice(128 * j, 128 * (j + 1))
                nc.tensor.matmul(atb_ps[:, 2 * j:2 * j + 2],
                                 alpha_row[:, cb], one_two[:],
                                 start=True, stop=True)
                nc.tensor.matmul(atb_ps[:, 16 + 2 * j:18 + 2 * j],
                                 beta2_row[:, cb], one_two[:],
                                 start=True, stop=True)
            nc.vector.tensor_copy(atb_sb[:], atb_ps[:, 0:32])

        fshard = feat_shard[:, :]
        outT_ap = outT_d[:, :]

        state = {}

        def prepare(b):
            """DMA y_b; im8 = y*alpha on DVE."""
            yb = ybp.tile([V, D], F32R, tag="yb", name=f"yb{b}")
            nc.sync.dma_start(yb[:], fshard[V * b: V * (b + 1), :])
            im8 = ybp.tile([V, D], F32R, tag="im8", name=f"im8_{b}")
            nc.vector.tensor_mul(im8[:], yb[:], alphaB[:, :].bitcast(F32R))
            rs8 = finp.tile([128, NBLK], fp32, tag="rs8", name=f"rs8_{b}")
            state[b] = [im8, None, [None] * NBLK, None, None, rs8]

        def prepare_xq(b):
            """xq = Wq@im8 + s(x)beta2 -> SBUF f32r."""
            im8 = state[b][0]
            xq_ps = pq_pool.tile([V, D], fp32, tag="pq", name=f"xq{b}")
            xg = xgp.tile([V, D], F32R, tag="xg", name=f"xg{b}")
            for h in range(2):
                c = slice(512 * h, 512 * (h + 1))
                nc.tensor.matmul(xq_ps[:, c], wq8_sb[:], im8[:, c],
                                 start=True, stop=False)
                nc.tensor.matmul(xq_ps[:, c], wqr8_sb[:], beta2_row[:, c],
                                 start=False, stop=True)
            nc.vector.tensor_copy(xg[:], xq_ps[:])
            state[b][1] = xg

        def prepare_xv(b):
            """xvT via 3 matmuls per block (beta2 + bias/ones rank-1 folds).
            sm layout: [0:80] xvT 10-col groups, [80:160] yT 10-col groups."""
            im8 = state[b][0]
            sm = ps_pool.tile([128, 512], fp32, tag="ps", name=f"sm{b}")
            for k in range(NBLK):
                dblk = slice(128 * k, 128 * (k + 1))
                g = slice(10 * k, 10 * (k + 1))
                nc.tensor.matmul(sm[:, g], im8[:, dblk], vw8_sb[:],
                                 start=True, stop=False)
                nc.tensor.matmul(sm[:, g], beta2_row[:, dblk], vwr8_sb[:],
                                 start=False, stop=False)
                nc.tensor.matmul(sm[:, g], onesD[:, dblk], vwr9_sb[:],
                                 start=False, stop=True)
            state[b][3] = sm

        rsx = const.tile([128, 2], fp32)

        def energy(b, k):
            xg = state[b][1]
            rs8 = state[b][5]
            dblk = slice(128 * k, 128 * (k + 1))
            pe = pe_pool.tile([128, D], fp32, tag="pe", name=f"pe{b}_{k}")
            esb = esbp.tile([128, D], F32R, tag="esb", name=f"esb{b}_{k}")
            if b == 0 and k < 2:
                # batch-0 head: half-width energy+exp so ACT starts on the
                # h0 chain without waiting for the h1 BN chain
                for h in range(2):
                    c = slice(512 * h, 512 * (h + 1))
                    nc.tensor.matmul(pe[:, c], xg[:, dblk], xg[:, c],
                                     start=True, stop=True)
                    acc = rs8[:, k:k + 1] if h == 0 else rsx[:, k:k + 1]
                    nc.scalar.activation(esb[:, c], pe[:, c], EXP,
                                         accum_out=acc)
                if k == 1:
                    nc.vector.tensor_add(rs8[:, 0:2], rs8[:, 0:2], rsx[:])
            else:
                for h in range(2):
                    c = slice(512 * h, 512 * (h + 1))
                    nc.tensor.matmul(pe[:, c], xg[:, dblk], xg[:, c],
                                     start=True, stop=True)
                nc.scalar.activation(esb[:], pe[:, :], EXP,
                                     accum_out=rs8[:, k:k + 1])
            state[b][2][k] = esb

        def bcast3(ap3, like3):
            from concourse.bass import broadcast_tensor_aps
            _, b3 = broadcast_tensor_aps(like3, ap3)
            return b3

        def xaug(b):
            """recip of rowsum; xaug = xvT * recip, one broadcast TT."""
            sm = state[b][3]
            rs8 = state[b][5]
            rr = finp.tile([128, NBLK], F32R, tag="rr", name=f"rr{b}")
            with nc.allow_low_precision(reason="f32r recip, small rel budget"):
                nc.vector.reciprocal(rr[:], rs8[:])
            xa = finp.tile([128, 10 * NBLK], F32R, tag="xa", name=f"xa{b}")
            xav = xa[:, :].rearrange("p (k c) -> p k c", c=10)
            smv = sm[:, 0:80].rearrange("p (k c) -> p k c", c=10)
            rrv = rr[:, :].bitcast(fp32).rearrange("p k -> p k ()")
            nc.vector.tensor_tensor(xav, smv, bcast3(rrv, xav), op=MULT)
            state[b][4] = xa

        def ymm(b, js):
            """yT_j[128, 10] += esb_kj^T @ xaug_k (col 8 = colsum')."""
            sm = state[b][3]
            xa = state[b][4]
            for j in js:
                eblk = slice(128 * j, 128 * (j + 1))
                for k in range(NBLK):
                    esb = state[b][2][k]
                    nc.tensor.matmul(sm[:, 80 + 10 * j: 90 + 10 * j],
                                     esb[:, eblk], xa[:, 10 * k: 10 * (k + 1)],
                                     start=(k == 0), stop=(k == NBLK - 1))

        def finalize(b, half=None):
            """s = 1/(1e-9+colsum'); osbT = yT*s + imgT; store transposed.
            half=0/1 processes j-blocks 0-3 / 4-7 (used to pipeline the
            final batch's renorm with its output DMA)."""
            sm = state[b][3]
            js = slice(0, NBLK) if half is None else slice(4 * half, 4 * half + 4)
            nj = js.stop - js.start
            if half in (None, 0):
                st = finp.tile([128, NBLK], fp32, tag="st", name=f"st{b}")
                s = finp.tile([128, NBLK], fp32, tag="s", name=f"s{b}")
                imgT = finp.tile([128, NBLK * V], fp32, tag="imgT",
                                 name=f"imgT{b}")
                osbT = finp.tile([128, NBLK * V], fp32, tag="osbT",
                                 name=f"osbT{b}")
                state[b].append((st, s, imgT, osbT))
            else:
                st, s, imgT, osbT = state[b][-1]
            cs_v = sm[:, 80:160].rearrange("p (j c) -> p j c", c=10)
            nc.vector.tensor_scalar_add(
                st[:, js].rearrange("p j -> p j ()"),
                cs_v[:, js, 8:9], 1e-9)
            nc.vector.reciprocal(s[:, js], st[:, js])
            fTv = featT_sb[:, 64 * b: 64 * (b + 1)].bitcast(fp32).rearrange(
                "p (j v) -> p j v", v=V)
            imv = imgT[:, :].rearrange("p (j v) -> p j v", v=V)
            aTv = atb_sb[:, 0:16].rearrange(
                "p (j c) -> p j c", c=2)[:, js, 0:1]
            bTv = atb_sb[:, 16:32].rearrange(
                "p (j c) -> p j c", c=2)[:, js, 0:1]
            nc.vector.tensor_tensor(imv[:, js, :], fTv[:, js, :],
                                    bcast3(aTv, imv[:, js, :]), op=MULT)
            nc.vector.tensor_tensor(imv[:, js, :], imv[:, js, :],
                                    bcast3(bTv, imv[:, js, :]), op=ADD)
            osv = osbT[:, :].rearrange("p (j v) -> p j v", v=V)
            sv3 = s[:, :].rearrange("p j -> p j ()")
            nc.vector.tensor_tensor(osv[:, js, :], cs_v[:, js, 0:8],
                                    bcast3(sv3[:, js, :], osv[:, js, :]),
                                    op=MULT)
            nc.vector.tensor_add(osv[:, js, :], osv[:, js, :], imv[:, js, :])
            nc.sync.dma_start(
                outT_ap[:, 64 * b + 8 * js.start: 64 * b + 8 * js.stop],
                osbT[:, 8 * js.start: 8 * js.stop])
            if half in (None, 1):
                state.pop(b)

        # ---- batch-0 h0-strict preamble: full h0 chain + h0 prep before
        # any h1 work, so the first exp isn't delayed by h1 chain ops ----
        im8_0 = ybp.tile([V, D], F32R, tag="im8", name="im8_0")
        xq_ps0 = pq_pool.tile([V, D], fp32, tag="pq", name="xq0")
        xg0 = xgp.tile([V, D], F32R, tag="xg", name="xg0")
        for h in range(2):
            c = slice(512 * h, 512 * (h + 1))
            chain_h(h)
            # im8 on Pool: DVE's startup stream stays the bare BN chain
            nc.gpsimd.tensor_tensor(im8_0[:, c], yb0[:, c],
                                    alphaB[:, c].bitcast(F32R), op=MULT)
            nc.tensor.matmul(xq_ps0[:, c], wq8_sb[:], im8_0[:, c],
                             start=True, stop=False)
            nc.tensor.matmul(xq_ps0[:, c], wqr8_sb[:], beta2_row[:, c],
                             start=False, stop=True)
            nc.vector.tensor_copy(xg0[:, c], xq_ps0[:, c])
        rs8_0 = finp.tile([128, NBLK], fp32, tag="rs8", name="rs8_0")
        state[0] = [im8_0, xg0, [None] * NBLK, None, None, rs8_0]

        # ---- software-pipelined main loop ----
        for b in range(BPC):
            nxt = b + 1 if b + 1 < BPC else None
            prv = b - 1 if b > 0 else None
            for k in range(NBLK):
                energy(b, k)
                if b == 0:
                    if k == 2:
                        prepare_xv(0)
                    elif k == 5:
                        build_atb()
                if prv is not None and 0 <= k <= 3:
                    ymm(prv, range(2 * k, 2 * k + 2))
                    if k == 3:
                        finalize(prv)
                if nxt is not None:
                    if k == 0:
                        prepare(nxt)
                    elif k == 4:
                        prepare_xq(nxt)
                    elif k == 5:
                        prepare_xv(nxt)
            xaug(b)
        last = BPC - 1
        ymm(last, range(4))
        finalize(last, half=0)
        ymm(last, range(4, NBLK))
        finalize(last, half=1)

    nc.compile()
    return nc


def _get():
    global _BUILT
    if _BUILT is None:
        _BUILT = _build_program()
    return _BUILT


def _make_in_maps(inputs):
    feat = np.ascontiguousarray(np.asarray(inputs["feat"], dtype=np.float32))
    gamma = np.asarray(inputs["bn_gamma"], dtype=np.float32).reshape(1, D)
    beta = np.asarray(inputs["bn_beta"], dtype=np.float32).reshape(1, D)
    qk = np.asarray(inputs["qk_weight"], dtype=np.float32)
    vw = np.asarray(inputs["v_weight"], dtype=np.float32)
    vb = np.asarray(inputs["v_bias"], dtype=np.float32)
    wq8 = np.ascontiguousarray(qk.T)
    wq_r8 = qk.sum(axis=1).reshape(1, V)
    # vw8T columns: 0-7 real x_v channels, 8 = constant-ones channel
    # (becomes the colsum' source after the recip scale), 9 = zero pad
    vw8 = np.zeros((V, 10), dtype=np.float32)
    vw8[:, 0:V] = vw.T
    vw_r8 = np.zeros((1, 10), dtype=np.float32)
    vw_r8[0, 0:V] = vw.sum(axis=1)
    vw_r9 = np.zeros((1, 10), dtype=np.float32)
    vw_r9[0, 0:V] = vb
    vw_r9[0, V] = 1.0
    full = np.ascontiguousarray(feat.reshape(ROWS, D))
    in_maps = []
    for c in range(NCORES):
        fc = feat[BPC * c: BPC * (c + 1)]              # [8, 8, 1024]
        shard = np.ascontiguousarray(fc.reshape(BPC * V, D))
        # featT[p, b*64 + j*8 + v] = fc[b, v, j*128 + p]
        ft4 = fc.reshape(BPC, V, NBLK, 128)            # b, v, j, p
        featT = np.ascontiguousarray(
            ft4.transpose(3, 0, 2, 1).reshape(128, BPC * NBLK * V))
        in_maps.append({
            "feat_full": full,
            "feat_shard": shard,
            "featT_shard": featT,
            "gamma": gamma,
            "beta": beta,
            "wq8T": wq8,
            "wq_r8": wq_r8,
            "vw8T": vw8,
            "vw_r8": vw_r8,
            "vw_r9": vw_r9,
        })
    return in_maps


def _run(inputs, **kw):
    from concourse.bass_utils import run_bass_kernel_spmd
    nc = _get()
    res = run_bass_kernel_spmd(nc, _make_in_maps(inputs),
                               core_ids=list(range(NCORES)), **kw)
    outs = []
    for c in range(NCORES):
        oT = res.results[c]["outT"]                    # [128, 512]
        o4 = oT.reshape(128, BPC, NBLK, V)             # p, b, j, v
        outs.append(o4.transpose(1, 3, 2, 0).reshape(BPC, V, D))
    return np.concatenate(outs, axis=0), res


def kernel(**inputs) -> np.ndarray:
    out, _ = _run(inputs)
    return out


def run_profiled(inputs, **kw):
    return _run(inputs, trace=True, **kw)
